# revision 11
# baseline (speedup 1.0000x reference)
"""Trainium2 Bass kernel for nn_Conditional_encoding (2-layer conditional LSTM encoder).

Data-parallel over 8 NeuronCores: batch 1024 -> 128 per core.

v2 design (per core, feature-major gates, BS=128 batch on the free axis):
  - Host precomputes PROJECTED gate tables proj_k = emb @ Wih_k.T + (bih_k+bhh_k)
    -> [V, 256] f32, gate column order [f|i|o|g]. One indirect row-gather per
    step fetches x's full gate contribution (bias included) batch-major.
  - PE transposes the gathered [128b, 256g] f32 into two gate PSUM tiles
    pg_fi=[f|i] and pg_og=[o|g] (start=True), then two bf16 matmuls
    accumulate Whh.T @ h_{t-1} on top (start=False). No ACT copies, no
    SBUF x staging, h-matmul contraction is K=64 only.
  - ACT per step: sigmoid(pg_fi) [128,128], tanh(g)=pg_og[64:128],
    sigmoid(o)=pg_og[0:64] (off critical path), tanh(c). Gate outputs bf16.
  - DVE per step: m2 = sf*C (hidden under tanh_g), m1 = si*tg (bf16 2x),
    C = m2+m1, h = so*tc (bf16 2x, ping-pong h tiles feed the next matmul).
  - take_along_axis gathers: per-step uint8 masks (host-built) +
    copy_predicated into accH/accC accumulators (off the chain).
  - Final MLP: two matmuls + Tanh-with-bias activation (as before).
"""

import sys

sys.path.insert(0, "/opt/trn_rl_repo")

import numpy as np

import concourse.bass as bass
import concourse.mybir as mybir
from concourse import tile
from concourse.bass_utils import run_bass_kernel_spmd

B, T1, T2, V, D, H = 1024, 256, 256, 32004, 50, 64
NCORES = 8
BS = B // NCORES  # 128 batch rows per core
F32 = mybir.dt.float32
BF16 = mybir.dt.bfloat16
I32 = mybir.dt.int32
U8 = mybir.dt.uint8
MULT = mybir.AluOpType.mult
ADD = mybir.AluOpType.add
TANH = mybir.ActivationFunctionType.Tanh
SIGM = mybir.ActivationFunctionType.Sigmoid


def _split_excess_waits(nc, max_waits=1):
    """Walrus CTRL lowering rejects multi-sem-wait instructions in this
    toolchain; move excess waits onto same-engine NOPs inserted before."""
    n_fixed = 0
    for f in nc.m.functions:
        for bb in f.blocks:
            insts = bb.instructions
            i = 0
            while i < len(insts):
                inst = insts[i]
                si = getattr(inst, "sync_info", None)
                if si is not None and si.on_wait and len(si.on_wait) > max_waits:
                    waits = list(si.on_wait)
                    si.on_wait = waits[-max_waits:]
                    excess = waits[:-max_waits]
                    pos = i
                    for j in range(0, len(excess), max_waits):
                        nop = mybir.InstNoOp(
                            name=f"{inst.name}-waitfix-{j}",
                            sync_info=mybir.SyncInfo(
                                on_wait=excess[j : j + max_waits], on_update=[]
                            ),
                            bass_nofuse=True,
                            engine=inst.engine,
                        )
                        insts.insert(pos, nop)
                        pos += 1
                        i += 1
                    n_fixed += 1
                i += 1
    return n_fixed


def build_program(t1=T1, t2=T2):
    nc = bass.Bass()
    p1d = nc.declare_dram_parameter("p1", [V, 256], F32, isOutput=False)
    p2d = nc.declare_dram_parameter("p2", [V, 256], F32, isOutput=False)
    s1d = nc.declare_dram_parameter("s1", [BS, t1], I32, isOutput=False)
    s2d = nc.declare_dram_parameter("s2", [BS, t2], I32, isOutput=False)
    # whh cols: 0:128 Whh1_fi | 128:256 Whh1_og | 256:384 Whh2_fi | 384:512 Whh2_og
    whhd = nc.declare_dram_parameter("whh", [64, 512], BF16, isOutput=False)
    identd = nc.declare_dram_parameter("ident", [128, 128], F32, isOutput=False)
    foldd = nc.declare_dram_parameter("fold", [128, 64], BF16, isOutput=False)
    # cf cols: 0:128 Wl1T (rows 0:64) | 128:132 Wl2T | 132 bl1 | 133 bl2 (rows 0:4)
    cfd = nc.declare_dram_parameter("cf", [128, 134], F32, isOutput=False)
    m1d = nc.declare_dram_parameter("m1", [64, t1 * BS], U8, isOutput=False)
    m2d = nc.declare_dram_parameter("m2", [64, t2 * BS], U8, isOutput=False)
    outd = nc.declare_dram_parameter("out", [4, BS], F32, isOutput=True)

    LA = 4  # x gather lookahead (steps)

    with tile.TileContext(nc) as tc:
        with (
            tc.tile_pool(name="const", bufs=1) as cpool,
            tc.tile_pool(name="state", bufs=1) as spool,
            tc.tile_pool(name="gx", bufs=LA + 3) as gxp,
            tc.tile_pool(name="pg", bufs=2, space="PSUM") as pgp,
            tc.tile_pool(name="cps", bufs=2, space="PSUM") as cpsp,
            tc.tile_pool(name="po", bufs=1, space="PSUM") as pop,
            tc.tile_pool(name="pz", bufs=1, space="PSUM") as pzp,
            tc.tile_pool(name="sfi", bufs=3) as sfip,
            tc.tile_pool(name="tg", bufs=3) as tgp,
            tc.tile_pool(name="mv", bufs=3) as mvp,
            tc.tile_pool(name="so", bufs=3) as sop,
            tc.tile_pool(name="tc", bufs=3) as tcp,
            tc.tile_pool(name="h", bufs=4) as hp,
        ):
            MSPLIT = 32
            mk1a = cpool.tile([64, MSPLIT * BS], U8)
            nc.sync.dma_start(out=mk1a[:], in_=m1d[:, 0 : MSPLIT * BS])
            sent1 = cpool.tile([BS, t1], I32)
            nc.sync.dma_start(out=sent1[:], in_=s1d[:])
            sent2 = cpool.tile([BS, t2], I32)
            nc.sync.dma_start(out=sent2[:], in_=s2d[:])
            whh = cpool.tile([64, 512], BF16)
            nc.sync.dma_start(out=whh[:], in_=whhd[:])
            ident = cpool.tile([128, 128], F32)
            nc.sync.dma_start(out=ident[:], in_=identd[:])
            fold = cpool.tile([128, 64], BF16)
            nc.sync.dma_start(out=fold[:], in_=foldd[:])
            cf = cpool.tile([128, 134], F32)
            nc.sync.dma_start(out=cf[:], in_=cfd[:])
            mk1b = cpool.tile([64, (t1 - MSPLIT) * BS], U8)
            nc.sync.dma_start(out=mk1b[:], in_=m1d[:, MSPLIT * BS :])
            mk2 = cpool.tile([64, t2 * BS], U8)
            nc.sync.dma_start(out=mk2[:], in_=m2d[:])

            def mk1(i):
                if i < MSPLIT:
                    return mk1a[:, i * BS : (i + 1) * BS]
                j = i - MSPLIT
                return mk1b[:, j * BS : (j + 1) * BS]

            def mk2f(i):
                return mk2[:, i * BS : (i + 1) * BS]

            Cm = spool.tile([64, BS], F32)  # SBUF mirror of the cell state
            nc.vector.memset(Cm[:], 0.0)
            accH = spool.tile([64, BS], F32)
            nc.vector.memset(accH[:], 0.0)
            accC = spool.tile([64, BS], F32)
            nc.vector.memset(accC[:], 0.0)
            accH2 = spool.tile([64, BS], F32)
            nc.vector.memset(accH2[:], 0.0)
            warm = spool.tile([64, 1], F32)
            nc.scalar.activation(warm[:], accH2[:, 0:1], SIGM)

            def prefetch_x(i, sent, proj):
                g = gxp.tile([BS, 256], F32, tag="gx")
                nc.gpsimd.indirect_dma_start(
                    out=g[:],
                    out_offset=None,
                    in_=proj[:],
                    in_offset=bass.IndirectOffsetOnAxis(ap=sent[:, i : i + 1], axis=0),
                )
                return g

            def lstm(t_steps, sent, proj, wfi, wog, mk, h0, acch, accc):
                gx = [prefetch_x(i, sent, proj) for i in range(min(LA, t_steps))]
                hprev = h0

                def emit_T(j):
                    pfi = pgp.tile([128, BS], F32, tag="pg_fi")
                    pog = pgp.tile([128, BS], F32, tag="pg_og")
                    nc.tensor.matmul(
                        pfi[:], gx[j][:, 0:128], ident[:],
                        is_transpose=True, start=True, stop=False,
                    )
                    nc.tensor.matmul(
                        pog[:], gx[j][:, 128:256], ident[:],
                        is_transpose=True, start=True, stop=False,
                    )
                    return pfi, pog

                pgs = [emit_T(0)]
                for i in range(t_steps):
                    pg_fi, pg_og = pgs[i]
                    nc.tensor.matmul(
                        pg_fi[:], wfi, hprev[:], start=False, stop=True
                    )
                    nc.tensor.matmul(
                        pg_og[:], wog, hprev[:], start=False, stop=True
                    )
                    if i + LA < t_steps:
                        gx.append(prefetch_x(i + LA, sent, proj))
                    sfi = sfip.tile([128, BS], BF16, tag="sfi")
                    nc.scalar.activation(sfi[:], pg_fi[:], SIGM)
                    W = tgp.tile([128, BS], BF16, tag="tg")
                    nc.scalar.activation(W[64:128, :], pg_og[64:128, :], TANH)
                    so = sop.tile([64, BS], BF16, tag="so")
                    nc.scalar.activation(so[:], pg_og[0:64, :], SIGM)
                    mv = mvp.tile([128, BS], BF16, tag="mv")
                    nc.vector.tensor_tensor(mv[0:64, :], sfi[0:64, :], Cm[:], MULT)
                    nc.vector.tensor_tensor(
                        mv[64:128, :], sfi[64:128, :], W[64:128, :], MULT
                    )
                    cps = cpsp.tile([64, BS], F32, tag="cps")
                    nc.tensor.matmul(cps[:], fold[:], mv[:])
                    if i + 1 < t_steps:
                        pgs.append(emit_T(i + 1))
                    tcl = tcp.tile([64, BS], BF16, tag="tc")
                    nc.scalar.activation(tcl[:], cps[:], TANH)
                    h = hp.tile([64, BS], BF16, tag="h")
                    nc.vector.tensor_tensor(h[:], so[:], tcl[:], MULT)
                    nc.vector.tensor_copy(Cm[:], cps[:])
                    m = mk(i)
                    nc.vector.copy_predicated(acch[:], m, h[:])
                    if accc is not None:
                        nc.vector.copy_predicated(accc[:], m, cps[:])
                    hprev = h

            # ---- LSTM1 ----
            h0 = hp.tile([64, BS], BF16, tag="h")
            nc.vector.memset(h0[:], 0.0)
            lstm(t1, sent1, p1d, whh[:, 0:128], whh[:, 128:256], mk1, h0,
                 accH, accC)

            # ---- LSTM2 init from gathered LSTM1 state ----
            h02 = hp.tile([64, BS], BF16, tag="h")
            nc.vector.tensor_copy(h02[:], accH[:])
            nc.vector.tensor_copy(Cm[:], accC[:])
            lstm(t2, sent2, p2d, whh[:, 256:384], whh[:, 384:512], mk2f, h02,
                 accH2, None)

            # ---- MLP head ----
            pz = pzp.tile([128, 128], F32, tag="pz")
            nc.tensor.matmul(pz[:], cf[0:64, 0:128], accH2[:])
            z1 = cpool.tile([128, 128], F32)
            nc.scalar.activation(z1[:], pz[:], TANH, bias=cf[:, 132:133])
            po = pop.tile([4, 128], F32, tag="po")
            nc.tensor.matmul(po[:], cf[:, 128:132], z1[:])
            o4 = cpool.tile([4, 128], F32)
            nc.scalar.add(o4[:], po[:], cf[0:4, 133:134])
            nc.sync.dma_start(out=outd[:], in_=o4[:])

    _split_excess_waits(nc)
    return nc


def make_proj(emb, Wih, bih, bhh):
    """[V, 256] f32 projected gate table, cols [f|i|o|g], bias folded in."""
    W = np.asarray(Wih, np.float32)  # [256, 50] rows: i,f,g,o blocks of 64
    b = np.asarray(bih, np.float32) + np.asarray(bhh, np.float32)
    e = np.asarray(emb, np.float32)  # [V, 50]
    out = np.empty((V, 256), np.float32)
    order = [1, 0, 3, 2]  # f, i, o, g  (pytorch blocks: i=0, f=1, g=2, o=3)
    for k, blk in enumerate(order):
        sl = slice(blk * 64, (blk + 1) * 64)
        out[:, k * 64 : (k + 1) * 64] = e @ W[sl].T + b[sl]
    return out


def make_whh(Whh1, Whh2):
    """[64, 512] bf16: per-LSTM [f|i] then [o|g] lhsT blocks."""
    out = np.empty((64, 512), np.float32)
    for li, W in enumerate([np.asarray(Whh1, np.float32),
                            np.asarray(Whh2, np.float32)]):
        base = li * 256
        out[:, base + 0 : base + 64] = W[64:128].T    # f
        out[:, base + 64 : base + 128] = W[0:64].T    # i
        out[:, base + 128 : base + 192] = W[192:256].T  # o
        out[:, base + 192 : base + 256] = W[128:192].T  # g
    return _to_bf16(out)


def build_masks(slen, t_steps):
    """slen: [BS, 64] int -> [64, t_steps * BS] uint8; m[j, t*BS+b] = (slen[b,j]==t)."""
    lt = np.ascontiguousarray(np.asarray(slen).T)  # [64, BS]
    eq = lt[:, None, :] == np.arange(t_steps, dtype=lt.dtype)[None, :, None]
    return np.ascontiguousarray(eq.reshape(64, -1).astype(np.uint8))


_prog_cache = {}


def get_program(t1=T1, t2=T2):
    key = (t1, t2)
    if key not in _prog_cache:
        _prog_cache[key] = build_program(t1, t2)
    return _prog_cache[key]


def make_in_maps(sentence1, sentence2, s1_len, s2_len, emb,
                 Wih1, Whh1, bih1, bhh1, Wih2, Whh2, bih2, bhh2,
                 Wl1, bl1, Wl2, bl2, t1=T1, t2=T2):
    p1 = make_proj(emb, Wih1, bih1, bhh1)
    p2 = make_proj(emb, Wih2, bih2, bhh2)
    whh = make_whh(Whh1, Whh2)
    ident = np.eye(128, dtype=np.float32)
    fold = _to_bf16(np.concatenate([np.eye(64, dtype=np.float32)] * 2, axis=0))

    cf = np.zeros((128, 134), np.float32)
    cf[0:64, 0:128] = np.asarray(Wl1, np.float32).T  # [64, 128]
    cf[:, 128:132] = np.asarray(Wl2, np.float32).T  # [128, 4]
    cf[:, 132] = np.asarray(bl1, np.float32)
    cf[0:4, 133] = np.asarray(bl2, np.float32)

    s1t = np.asarray(sentence1, np.int32)
    s2t = np.asarray(sentence2, np.int32)
    l1 = np.asarray(s1_len, np.int64)[:, 0, :]  # [B, 64]
    l2 = np.asarray(s2_len, np.int64)[:, 0, :]

    in_maps = []
    for c in range(NCORES):
        sl = slice(c * BS, (c + 1) * BS)
        in_maps.append({
            "p1": p1,
            "p2": p2,
            "s1": np.ascontiguousarray(s1t[sl, :t1]),
            "s2": np.ascontiguousarray(s2t[sl, :t2]),
            "whh": whh,
            "ident": ident,
            "fold": fold,
            "cf": cf,
            "m1": build_masks(l1[sl], t1),
            "m2": build_masks(l2[sl], t2),
        })
    return in_maps


def _to_bf16(a):
    try:
        import ml_dtypes

        return a.astype(ml_dtypes.bfloat16)
    except ImportError:
        u = a.astype(np.float32).view(np.uint32)
        rounded = ((u + 0x7FFF + ((u >> 16) & 1)) >> 16).astype(np.uint16)
        return rounded


def kernel(sentence1, sentence2, s1_len, s2_len, s1_s, s2_s, emb,
           Wih1, Whh1, bih1, bhh1, Wih2, Whh2, bih2, bhh2,
           Wl1, bl1, Wl2, bl2):
    nc = get_program()
    in_maps = make_in_maps(sentence1, sentence2, s1_len, s2_len, emb,
                           Wih1, Whh1, bih1, bhh1, Wih2, Whh2, bih2, bhh2,
                           Wl1, bl1, Wl2, bl2)
    res = run_bass_kernel_spmd(nc, in_maps, list(range(NCORES)))
    out = np.zeros((B, 4), np.float32)
    for c in range(NCORES):
        out[c * BS : (c + 1) * BS, :] = res.results[c]["out"].T
    return out


# revision 12
# speedup vs baseline: 1.0636x; 1.0636x over previous
"""Trainium2 Bass kernel for nn_Conditional_encoding (2-layer conditional LSTM encoder).

Data-parallel over 8 NeuronCores: batch 1024 -> 128 per core.

Design (per core, feature-major gates, BS=128 batch on the free axis):
  - Host precomputes PROJECTED gate tables proj_k = emb @ Wih_k.T + (bih_k+bhh_k)
    -> [V, 256] f32, gate column order [f|i|o|g]. One indirect row-gather per
    step fetches x's full gate contribution (bias included) batch-major.
  - PE transposes the gathered [128b, 256g] f32 into two gate PSUM tiles
    pg_fi=[f|i] and pg_og=[o|g] (start=True), then two bf16 matmuls
    accumulate Whh.T @ h_{t-1} on top (start=False). No ACT copies, no
    SBUF x staging, h-matmul contraction is K=64 only.
  - ACT per step: sigmoid(pg_fi) [128,128], tanh(g) -> W[64:128] (base-64
    aligned for the DVE), sigmoid(o) (off critical path), tanh(cps).
    Gate outputs bf16.
  - DVE products into one [128,BS] bf16 tile: mv[0:64] = sf*Cm (hidden under
    tanh_g), mv[64:128] = si*tg (bf16 2x). Walrus requires same start
    partition for all DVE operands, so the cross-partition fold
    c' = mv0 + mv1 is a tiny PE matmul with a stacked-identity lhsT,
    accumulating exactly in fp32 PSUM (cps). tanh(c) reads the PSUM
    directly; Cm (SBUF fp32 mirror for the next step's m2) is refreshed
    off the critical path after hmult. h = so*tc (bf16 2x), ping-pong
    h tiles feed the next step's matmul.
  - take_along_axis gathers: per-step uint8 masks (host-built) +
    copy_predicated into accH/accC accumulators (off the chain). The
    LSTM1 mask upload is split so the first 32 steps' slice arrives
    before step 0 needs it; a dummy sigmoid preloads the ACT table set.
  - Final MLP: two matmuls + Tanh-with-bias activation.

CoreSim: ~0.90 ms/core (baseline ~1.28 ms); chain ~1.65 us/step =
sigmoid_fi + tanh_g + m1 + fold-MM + tanh_c + hmult + 4 sem hops.
"""

import sys

sys.path.insert(0, "/opt/trn_rl_repo")

import numpy as np

import concourse.bass as bass
import concourse.mybir as mybir
from concourse import tile
from concourse.bass_utils import run_bass_kernel_spmd

B, T1, T2, V, D, H = 1024, 256, 256, 32004, 50, 64
NCORES = 8
BS = B // NCORES  # 128 batch rows per core
F32 = mybir.dt.float32
BF16 = mybir.dt.bfloat16
I32 = mybir.dt.int32
U8 = mybir.dt.uint8
MULT = mybir.AluOpType.mult
ADD = mybir.AluOpType.add
TANH = mybir.ActivationFunctionType.Tanh
SIGM = mybir.ActivationFunctionType.Sigmoid


def _split_excess_waits(nc, max_waits=1):
    """Walrus CTRL lowering rejects multi-sem-wait instructions in this
    toolchain; move excess waits onto same-engine NOPs inserted before."""
    n_fixed = 0
    for f in nc.m.functions:
        for bb in f.blocks:
            insts = bb.instructions
            i = 0
            while i < len(insts):
                inst = insts[i]
                si = getattr(inst, "sync_info", None)
                if si is not None and si.on_wait and len(si.on_wait) > max_waits:
                    waits = list(si.on_wait)
                    si.on_wait = waits[-max_waits:]
                    excess = waits[:-max_waits]
                    pos = i
                    for j in range(0, len(excess), max_waits):
                        nop = mybir.InstNoOp(
                            name=f"{inst.name}-waitfix-{j}",
                            sync_info=mybir.SyncInfo(
                                on_wait=excess[j : j + max_waits], on_update=[]
                            ),
                            bass_nofuse=True,
                            engine=inst.engine,
                        )
                        insts.insert(pos, nop)
                        pos += 1
                        i += 1
                    n_fixed += 1
                i += 1
    return n_fixed


def build_program(t1=T1, t2=T2):
    nc = bass.Bass()
    p1d = nc.declare_dram_parameter("p1", [V, 256], F32, isOutput=False)
    p2d = nc.declare_dram_parameter("p2", [V, 256], F32, isOutput=False)
    s1d = nc.declare_dram_parameter("s1", [BS, t1], I32, isOutput=False)
    s2d = nc.declare_dram_parameter("s2", [BS, t2], I32, isOutput=False)
    # whh cols: 0:128 Whh1_fi | 128:256 Whh1_og | 256:384 Whh2_fi | 384:512 Whh2_og
    whhd = nc.declare_dram_parameter("whh", [64, 512], BF16, isOutput=False)
    identd = nc.declare_dram_parameter("ident", [128, 128], F32, isOutput=False)
    foldd = nc.declare_dram_parameter("fold", [128, 64], BF16, isOutput=False)
    # cf cols: 0:128 Wl1T (rows 0:64) | 128:132 Wl2T | 132 bl1 | 133 bl2 (rows 0:4)
    cfd = nc.declare_dram_parameter("cf", [128, 134], F32, isOutput=False)
    m1d = nc.declare_dram_parameter("m1", [64, t1 * BS], U8, isOutput=False)
    m2d = nc.declare_dram_parameter("m2", [64, t2 * BS], U8, isOutput=False)
    outd = nc.declare_dram_parameter("out", [4, BS], F32, isOutput=True)

    LA = 4  # x gather lookahead (steps)

    with tile.TileContext(nc) as tc:
        with (
            tc.tile_pool(name="const", bufs=1) as cpool,
            tc.tile_pool(name="state", bufs=1) as spool,
            tc.tile_pool(name="gx", bufs=LA + 3) as gxp,
            tc.tile_pool(name="pg", bufs=2, space="PSUM") as pgp,
            tc.tile_pool(name="cps", bufs=2, space="PSUM") as cpsp,
            tc.tile_pool(name="po", bufs=1, space="PSUM") as pop,
            tc.tile_pool(name="pz", bufs=1, space="PSUM") as pzp,
            tc.tile_pool(name="sfi", bufs=3) as sfip,
            tc.tile_pool(name="tg", bufs=3) as tgp,
            tc.tile_pool(name="mv", bufs=3) as mvp,
            tc.tile_pool(name="so", bufs=3) as sop,
            tc.tile_pool(name="tc", bufs=3) as tcp,
            tc.tile_pool(name="h", bufs=4) as hp,
        ):
            MSPLIT = 32
            mk1a = cpool.tile([64, MSPLIT * BS], U8)
            nc.sync.dma_start(out=mk1a[:], in_=m1d[:, 0 : MSPLIT * BS])
            sent1 = cpool.tile([BS, t1], I32)
            nc.sync.dma_start(out=sent1[:], in_=s1d[:])
            sent2 = cpool.tile([BS, t2], I32)
            nc.sync.dma_start(out=sent2[:], in_=s2d[:])
            whh = cpool.tile([64, 512], BF16)
            nc.sync.dma_start(out=whh[:], in_=whhd[:])
            ident = cpool.tile([128, 128], F32)
            nc.sync.dma_start(out=ident[:], in_=identd[:])
            fold = cpool.tile([128, 64], BF16)
            nc.sync.dma_start(out=fold[:], in_=foldd[:])
            cf = cpool.tile([128, 134], F32)
            nc.sync.dma_start(out=cf[:], in_=cfd[:])
            mk1b = cpool.tile([64, (t1 - MSPLIT) * BS], U8)
            nc.sync.dma_start(out=mk1b[:], in_=m1d[:, MSPLIT * BS :])
            mk2 = cpool.tile([64, t2 * BS], U8)
            nc.sync.dma_start(out=mk2[:], in_=m2d[:])

            def mk1(i):
                if i < MSPLIT:
                    return mk1a[:, i * BS : (i + 1) * BS]
                j = i - MSPLIT
                return mk1b[:, j * BS : (j + 1) * BS]

            def mk2f(i):
                return mk2[:, i * BS : (i + 1) * BS]

            Cm = spool.tile([64, BS], F32)  # SBUF mirror of the cell state
            nc.vector.memset(Cm[:], 0.0)
            accH = spool.tile([64, BS], F32)
            nc.vector.memset(accH[:], 0.0)
            accC = spool.tile([64, BS], F32)
            nc.vector.memset(accC[:], 0.0)
            accH2 = spool.tile([64, BS], F32)
            nc.vector.memset(accH2[:], 0.0)
            warm = spool.tile([64, 1], F32)
            nc.scalar.activation(warm[:], accH2[:, 0:1], SIGM)

            def prefetch_x(i, sent, proj):
                g = gxp.tile([BS, 256], F32, tag="gx")
                nc.gpsimd.indirect_dma_start(
                    out=g[:],
                    out_offset=None,
                    in_=proj[:],
                    in_offset=bass.IndirectOffsetOnAxis(ap=sent[:, i : i + 1], axis=0),
                )
                return g

            def lstm(t_steps, sent, proj, wfi, wog, mk, h0, acch, accc):
                gx = [prefetch_x(i, sent, proj) for i in range(min(LA, t_steps))]
                hprev = h0

                def emit_T(j):
                    pfi = pgp.tile([128, BS], F32, tag="pg_fi")
                    pog = pgp.tile([128, BS], F32, tag="pg_og")
                    nc.tensor.matmul(
                        pfi[:], gx[j][:, 0:128], ident[:],
                        is_transpose=True, start=True, stop=False,
                    )
                    nc.tensor.matmul(
                        pog[:], gx[j][:, 128:256], ident[:],
                        is_transpose=True, start=True, stop=False,
                    )
                    return pfi, pog

                pgs = [emit_T(0)]
                for i in range(t_steps):
                    pg_fi, pg_og = pgs[i]
                    nc.tensor.matmul(
                        pg_fi[:], wfi, hprev[:], start=False, stop=True
                    )
                    nc.tensor.matmul(
                        pg_og[:], wog, hprev[:], start=False, stop=True
                    )
                    if i + LA < t_steps:
                        gx.append(prefetch_x(i + LA, sent, proj))
                    sfi = sfip.tile([128, BS], BF16, tag="sfi")
                    nc.scalar.activation(sfi[:], pg_fi[:], SIGM)
                    W = tgp.tile([128, BS], BF16, tag="tg")
                    nc.scalar.activation(W[64:128, :], pg_og[64:128, :], TANH)
                    so = sop.tile([64, BS], BF16, tag="so")
                    nc.scalar.activation(so[:], pg_og[0:64, :], SIGM)
                    mv = mvp.tile([128, BS], BF16, tag="mv")
                    nc.vector.tensor_tensor(mv[0:64, :], sfi[0:64, :], Cm[:], MULT)
                    nc.vector.tensor_tensor(
                        mv[64:128, :], sfi[64:128, :], W[64:128, :], MULT
                    )
                    cps = cpsp.tile([64, BS], F32, tag="cps")
                    nc.tensor.matmul(cps[:], fold[:], mv[:])
                    if i + 1 < t_steps:
                        pgs.append(emit_T(i + 1))
                    tcl = tcp.tile([64, BS], BF16, tag="tc")
                    nc.scalar.activation(tcl[:], cps[:], TANH)
                    h = hp.tile([64, BS], BF16, tag="h")
                    nc.vector.tensor_tensor(h[:], so[:], tcl[:], MULT)
                    nc.vector.tensor_copy(Cm[:], cps[:])
                    m = mk(i)
                    nc.vector.copy_predicated(acch[:], m, h[:])
                    if accc is not None:
                        nc.vector.copy_predicated(accc[:], m, cps[:])
                    hprev = h

            # ---- LSTM1 ----
            h0 = hp.tile([64, BS], BF16, tag="h")
            nc.vector.memset(h0[:], 0.0)
            lstm(t1, sent1, p1d, whh[:, 0:128], whh[:, 128:256], mk1, h0,
                 accH, accC)

            # ---- LSTM2 init from gathered LSTM1 state ----
            h02 = hp.tile([64, BS], BF16, tag="h")
            nc.vector.tensor_copy(h02[:], accH[:])
            nc.vector.tensor_copy(Cm[:], accC[:])
            lstm(t2, sent2, p2d, whh[:, 256:384], whh[:, 384:512], mk2f, h02,
                 accH2, None)

            # ---- MLP head ----
            pz = pzp.tile([128, 128], F32, tag="pz")
            nc.tensor.matmul(pz[:], cf[0:64, 0:128], accH2[:])
            z1 = cpool.tile([128, 128], F32)
            nc.scalar.activation(z1[:], pz[:], TANH, bias=cf[:, 132:133])
            po = pop.tile([4, 128], F32, tag="po")
            nc.tensor.matmul(po[:], cf[:, 128:132], z1[:])
            o4 = cpool.tile([4, 128], F32)
            nc.scalar.add(o4[:], po[:], cf[0:4, 133:134])
            nc.sync.dma_start(out=outd[:], in_=o4[:])

    _split_excess_waits(nc)
    return nc


def make_proj(emb, Wih, bih, bhh):
    """[V, 256] f32 projected gate table, cols [f|i|o|g], bias folded in."""
    W = np.asarray(Wih, np.float32)  # [256, 50] rows: i,f,g,o blocks of 64
    b = np.asarray(bih, np.float32) + np.asarray(bhh, np.float32)
    e = np.asarray(emb, np.float32)  # [V, 50]
    out = np.empty((V, 256), np.float32)
    order = [1, 0, 3, 2]  # f, i, o, g  (pytorch blocks: i=0, f=1, g=2, o=3)
    for k, blk in enumerate(order):
        sl = slice(blk * 64, (blk + 1) * 64)
        out[:, k * 64 : (k + 1) * 64] = e @ W[sl].T + b[sl]
    return out


def make_whh(Whh1, Whh2):
    """[64, 512] bf16: per-LSTM [f|i] then [o|g] lhsT blocks."""
    out = np.empty((64, 512), np.float32)
    for li, W in enumerate([np.asarray(Whh1, np.float32),
                            np.asarray(Whh2, np.float32)]):
        base = li * 256
        out[:, base + 0 : base + 64] = W[64:128].T    # f
        out[:, base + 64 : base + 128] = W[0:64].T    # i
        out[:, base + 128 : base + 192] = W[192:256].T  # o
        out[:, base + 192 : base + 256] = W[128:192].T  # g
    return _to_bf16(out)


def build_masks(slen, t_steps):
    """slen: [BS, 64] int -> [64, t_steps * BS] uint8; m[j, t*BS+b] = (slen[b,j]==t)."""
    lt = np.ascontiguousarray(np.asarray(slen).T)  # [64, BS]
    eq = lt[:, None, :] == np.arange(t_steps, dtype=lt.dtype)[None, :, None]
    return np.ascontiguousarray(eq.reshape(64, -1).astype(np.uint8))


_prog_cache = {}


def get_program(t1=T1, t2=T2):
    key = (t1, t2)
    if key not in _prog_cache:
        _prog_cache[key] = build_program(t1, t2)
    return _prog_cache[key]


def make_in_maps(sentence1, sentence2, s1_len, s2_len, emb,
                 Wih1, Whh1, bih1, bhh1, Wih2, Whh2, bih2, bhh2,
                 Wl1, bl1, Wl2, bl2, t1=T1, t2=T2):
    p1 = make_proj(emb, Wih1, bih1, bhh1)
    p2 = make_proj(emb, Wih2, bih2, bhh2)
    whh = make_whh(Whh1, Whh2)
    ident = np.eye(128, dtype=np.float32)
    fold = _to_bf16(np.concatenate([np.eye(64, dtype=np.float32)] * 2, axis=0))

    cf = np.zeros((128, 134), np.float32)
    cf[0:64, 0:128] = np.asarray(Wl1, np.float32).T  # [64, 128]
    cf[:, 128:132] = np.asarray(Wl2, np.float32).T  # [128, 4]
    cf[:, 132] = np.asarray(bl1, np.float32)
    cf[0:4, 133] = np.asarray(bl2, np.float32)

    s1t = np.asarray(sentence1, np.int32)
    s2t = np.asarray(sentence2, np.int32)
    l1 = np.asarray(s1_len, np.int64)[:, 0, :]  # [B, 64]
    l2 = np.asarray(s2_len, np.int64)[:, 0, :]

    in_maps = []
    for c in range(NCORES):
        sl = slice(c * BS, (c + 1) * BS)
        in_maps.append({
            "p1": p1,
            "p2": p2,
            "s1": np.ascontiguousarray(s1t[sl, :t1]),
            "s2": np.ascontiguousarray(s2t[sl, :t2]),
            "whh": whh,
            "ident": ident,
            "fold": fold,
            "cf": cf,
            "m1": build_masks(l1[sl], t1),
            "m2": build_masks(l2[sl], t2),
        })
    return in_maps


def _to_bf16(a):
    try:
        import ml_dtypes

        return a.astype(ml_dtypes.bfloat16)
    except ImportError:
        u = a.astype(np.float32).view(np.uint32)
        rounded = ((u + 0x7FFF + ((u >> 16) & 1)) >> 16).astype(np.uint16)
        return rounded


def kernel(sentence1, sentence2, s1_len, s2_len, s1_s, s2_s, emb,
           Wih1, Whh1, bih1, bhh1, Wih2, Whh2, bih2, bhh2,
           Wl1, bl1, Wl2, bl2):
    nc = get_program()
    in_maps = make_in_maps(sentence1, sentence2, s1_len, s2_len, emb,
                           Wih1, Whh1, bih1, bhh1, Wih2, Whh2, bih2, bhh2,
                           Wl1, bl1, Wl2, bl2)
    res = run_bass_kernel_spmd(nc, in_maps, list(range(NCORES)))
    out = np.zeros((B, 4), np.float32)
    for c in range(NCORES):
        out[c * BS : (c + 1) * BS, :] = res.results[c]["out"].T
    return out
